# revision 20
# baseline (speedup 1.0000x reference)
"""Graphormer attention (N=2048, D=512, H=8 heads of 64) on 8 NeuronCores.

Strategy (tensor-parallel over heads, one head per core):
  - Host slices Q/K/V/O projection weights per head, transposes x once.
  - The z-bin bias is folded in multiplicatively: the per-head bias table is
    tiny (16 entries), so host precomputes W = exp(z_table[bin(z)]) transposed
    to the kernel's [key, query] layout, shipped as bf16.
  - On device (per core): Q^T/K^T/V projections, S^T = K^T-tiles x Q^T
    (fp32, PSUM), exp on ScalarE -> bf16, P = exp(S) * W on VectorE,
    O'^T = sum_k V'[k,65] x P (65th V column = ones => row 64 of O' is the
    softmax denominator Z), then Y^T = Wo_h^T-tiles x O^T.
  - Host divides each head's partial Y by its Z, sums heads, adds bias terms.
"""

import numpy as np
import ml_dtypes
from contextlib import ExitStack

import concourse.bass as bass
import concourse.tile as tile
from concourse import bacc, mybir
from concourse import bass_utils

N = 2048
D = 512
H = 8
HD = 64
NUM_Z_BINS = 16
MAX_Z = 5.0
SCALE = HD ** -0.5
NCORES = 8
QL = 1024          # query-chunk length (PSUM budget)
QC = N // QL       # 2 query chunks
KT = N // 128      # 16 key tiles

FP32 = mybir.dt.float32
FP16 = mybir.dt.float16
BF16 = mybir.dt.bfloat16
BF16_NP = ml_dtypes.bfloat16
FP16_NP = np.float16

AF = mybir.ActivationFunctionType
OP = mybir.AluOpType

_PROGRAM_CACHE = {}


def _build_program():
    if "nc" in _PROGRAM_CACHE:
        return _PROGRAM_CACHE["nc"]

    nc = bacc.Bacc(
        "TRN2",
        target_bir_lowering=False,
        debug=False,
        enable_asserts=False,
        num_devices=NCORES,
    )

    xT = nc.dram_tensor("xT", [D, N], BF16, kind="ExternalInput").ap()
    wqk = nc.dram_tensor("wqk", [D, 128], BF16, kind="ExternalInput").ap()
    wv = nc.dram_tensor("wv", [D, HD], BF16, kind="ExternalInput").ap()
    wo = nc.dram_tensor("wo", [HD, D], FP16, kind="ExternalInput").ap()
    bqk = nc.dram_tensor("bqk", [128], FP32, kind="ExternalInput").ap()
    sclv = nc.dram_tensor("sclv", [128], FP32, kind="ExternalInput").ap()
    wt = nc.dram_tensor("wt", [N, N], FP16, kind="ExternalInput").ap()

    ypT = nc.dram_tensor("ypT", [D, N], FP16, kind="ExternalOutput").ap()
    zrow = nc.dram_tensor("zrow", [N], FP16, kind="ExternalOutput").ap()

    with tile.TileContext(nc) as tc:
        with ExitStack() as ctx:
            _emit(ctx, tc, xT, wqk, wv, wo, bqk, sclv, wt, ypT, zrow)
    nc.compile()
    _PROGRAM_CACHE["nc"] = nc
    return nc


def _emit(ctx, tc, xT, wqk, wv, wo, bqk, sclv, wt, ypT, zrow):
    nc = tc.nc
    CH = D // 128  # 4 contraction chunks of the model dim

    singles = ctx.enter_context(tc.tile_pool(name="singles", bufs=1))
    # PSUM budget is 16KB/partition (8 banks). ps_a slots are [128,1024]fp32
    # (4KB/part, 3 slots = 6 banks) shared by the qk/v/s/y phases; ps_o (2
    # banks) holds the O' accumulator.
    ps_a = ctx.enter_context(tc.tile_pool(name="ps_a", bufs=3, space="PSUM"))
    ps_o = ctx.enter_context(tc.tile_pool(name="ps_o", bufs=1, space="PSUM"))
    wpool = ctx.enter_context(tc.tile_pool(name="wpool", bufs=2))
    epool = ctx.enter_context(tc.tile_pool(name="epool", bufs=3))
    ppool = ctx.enter_context(tc.tile_pool(name="ppool", bufs=3))
    ypool = ctx.enter_context(tc.tile_pool(name="ypool", bufs=4))

    # ---- load constants -------------------------------------------------
    # column-chunked x^T: tile j holds columns [j*512,(j+1)*512) for every
    # 128-row contraction chunk, so compute can start after the first chunk.
    xr = xT.rearrange("(c p) (j n) -> j c p n", p=128, n=512)
    xT_cc = []
    for j in range(N // 512):
        t_ = singles.tile([128, CH * 512], BF16, tag=f"xTc{j}")
        for c in range(CH):
            nc.sync.dma_start(out=t_[:, c * 512:(c + 1) * 512], in_=xr[j, c])
        xT_cc.append(t_)
    # PE warm-up: dummy matmuls on never-written scratch keep the HAM busy
    # (and warm) while the x^T DMA streams in.
    scratch = singles.tile([128, 512], BF16)
    nc.vector.memset(scratch, 0.0)
    wu = ps_a.tile([128, 512], FP32, tag="big")
    for _ in range(16):
        nc.tensor.matmul(wu, lhsT=scratch[:, 0:128], rhs=scratch,
                         start=True, stop=True)
    wqk_sb = singles.tile([128, CH * 128], BF16)
    wv_sb = singles.tile([128, CH * HD], BF16)
    for c in range(CH):
        nc.sync.dma_start(out=wqk_sb[:, c * 128:(c + 1) * 128],
                          in_=wqk.rearrange("(c p) m -> c p m", p=128)[c])
        nc.sync.dma_start(out=wv_sb[:, c * HD:(c + 1) * HD],
                          in_=wv.rearrange("(c p) m -> c p m", p=128)[c])
    wo_sb = singles.tile([HD, D], FP16)
    nc.sync.dma_start(out=wo_sb, in_=wo)
    bqk_sb = singles.tile([128, 1], FP32)
    nc.sync.dma_start(out=bqk_sb, in_=bqk.rearrange("(n a) -> n a", a=1))

    # ---- fused Q^T/K^T projection: one [128,128] weight block computes
    # Q^T into PSUM rows 0-63 and K^T into rows 64-127 (full PE array).
    qT_sb = singles.tile([HD, N], BF16)
    kT_sb = singles.tile([HD, N], BF16)
    for half in range(N // 1024):
        pt = ps_a.tile([128, 1024], FP32, tag="big")
        for n in range(2):
            j = half * 2 + n
            for c in range(CH):
                nc.tensor.matmul(
                    pt[:, n * 512:(n + 1) * 512],
                    lhsT=wqk_sb[:, c * 128:(c + 1) * 128],
                    rhs=xT_cc[j][:, c * 512:(c + 1) * 512],
                    start=(c == 0),
                    stop=(c == CH - 1),
                )
        dst = slice(half * 1024, (half + 1) * 1024)
        nc.vector.tensor_scalar(kT_sb[:, dst], pt[HD:128, :], bqk_sb[HD:128, :],
                                None, OP.add)
        nc.vector.tensor_scalar(qT_sb[:, dst], pt[0:HD, :], bqk_sb[0:HD, :],
                                SCALE, OP.add, OP.mult)

    # ---- main loop: S^T -> exp -> *W -> PV, software-pipelined ----------
    # PV(t) depends on exp/mult of t; emitting S(t+1) BEFORE PV(t) keeps the
    # tensor engine's in-order queue from stalling on the ACT/DVE chain.
    # Each qc's Y projection + output DMA overlaps the next qc's loop.
    v_sb = singles.tile([128, KT * (HD + 1)], FP16)
    oT_sb = singles.tile([HD + 1, N], FP16)
    for qc in range(QC):
        ot = ps_o.tile([HD + 1, QL], FP32, tag="ot")
        pending = {}

        def emit_s(t, qc=qc, pending=pending):
            st = ps_a.tile([128, QL], FP32, tag="big")
            w_tile = wpool.tile([128, QL], FP16, tag="w")
            nc.sync.dma_start(
                out=w_tile,
                in_=wt[t * 128:(t + 1) * 128, qc * QL:(qc + 1) * QL],
            )
            for n in range(QL // 512):
                nc.tensor.matmul(
                    st[:, n * 512:(n + 1) * 512],
                    lhsT=kT_sb[:, t * 128:(t + 1) * 128],
                    rhs=qT_sb[:, qc * QL + n * 512: qc * QL + (n + 1) * 512],
                    start=True,
                    stop=True,
                )
            pending[t] = (st, w_tile)

        emit_s(0)
        emit_s(1)
        if qc == 0:
            # V' : [k-tile 128, 65] per tile, col 64 = 1.0. Emitted after the
            # first S tile so the first exp starts as early as possible.
            nc.vector.memset(v_sb, 1.0)
            vp = ps_a.tile([128, KT * HD], FP32, tag="big")
            for m in range(KT):
                for c in range(CH):
                    nc.tensor.matmul(
                        vp[:, m * HD:(m + 1) * HD],
                        lhsT=xT_cc[m // 4][:, c * 512 + (m % 4) * 128:
                                           c * 512 + (m % 4) * 128 + 128],
                        rhs=wv_sb[:, c * HD:(c + 1) * HD],
                        start=(c == 0),
                        stop=(c == CH - 1),
                    )
            nc.vector.tensor_copy(
                v_sb.rearrange("p (t c) -> p t c", c=HD + 1)[:, :, 0:HD],
                vp.rearrange("p (t c) -> p t c", c=HD),
            )
        for t in range(KT):
            if t + 2 < KT:
                emit_s(t + 2)
            st, w_tile = pending.pop(t)
            e_tile = epool.tile([128, QL], FP16, tag="e")
            nc.scalar.activation(e_tile, st, AF.Exp)
            p_tile = ppool.tile([128, QL], FP16, tag="p")
            nc.vector.tensor_mul(p_tile, e_tile, w_tile)
            for n in range(QL // 512):
                nc.tensor.matmul(
                    ot[:, n * 512:(n + 1) * 512],
                    lhsT=v_sb[:, t * (HD + 1):(t + 1) * (HD + 1)],
                    rhs=p_tile[:, n * 512:(n + 1) * 512],
                    start=(t == 0),
                    stop=(t == KT - 1),
                )
        nc.scalar.copy(oT_sb[:, qc * QL:(qc + 1) * QL], ot)

    nc.sync.dma_start(out=zrow.rearrange("(a n) -> a n", a=1),
                      in_=oT_sb[HD:HD + 1, :])

    # ---- tail: Y^T = Wo^T-tiles x O^T, evacs split across DVE/ACT -------
    for n2 in range(N // 1024):
        for m in range(D // 128):
            yt = ps_a.tile([128, 1024], FP32, tag="big")
            for nl in range(2):
                n = n2 * 2 + nl
                nc.tensor.matmul(
                    yt[:, nl * 512:(nl + 1) * 512],
                    lhsT=wo_sb[:, m * 128:(m + 1) * 128],
                    rhs=oT_sb[0:HD, n * 512:(n + 1) * 512],
                    start=True,
                    stop=True,
                )
            y_sb = ypool.tile([128, 1024], FP16, tag="ysb")
            if m % 2 == 0:
                nc.vector.tensor_copy(y_sb, yt)
            else:
                nc.scalar.copy(y_sb, yt)
            nc.sync.dma_start(
                out=ypT[m * 128:(m + 1) * 128, n2 * 1024:(n2 + 1) * 1024],
                in_=y_sb,
            )


def _install_ntff_hook():
    """Recreate the missing ``antenv.axon_hooks`` module so that
    run_bass_kernel_spmd(trace=True) can capture NTFF profiles via the
    libaxon_pjrt.so ctypes hook (see trn_agent_boot.trn_boot)."""
    import sys
    import types

    try:
        import antenv.axon_hooks  # noqa: F401
        return
    except ImportError:
        pass
    import antenv
    from trn_agent_boot.trn_boot import _ntff_profile_via_ctypes

    mod = types.ModuleType("antenv.axon_hooks")
    mod._hook = _ntff_profile_via_ctypes("/opt/axon/libaxon_pjrt.so")
    mod.set_axon_ntff_profile_hook = lambda h: setattr(mod, "_hook", h)
    mod.get_axon_ntff_profile_hook = lambda: mod._hook
    sys.modules["antenv.axon_hooks"] = mod
    antenv.axon_hooks = mod
    # keep profile artifacts local; the sandbox has no bucket access
    bass_utils.upload_artifacts = lambda tmpdir: tmpdir


def kernel(x, z_matrix, Wq, bq, Wk, bk, Wv, bv, Wo, bo, z_table, _trace=False):
    if _trace:
        _install_ntff_hook()
    x = np.ascontiguousarray(np.asarray(x, dtype=np.float32))
    z_matrix = np.asarray(z_matrix, dtype=np.float32)
    Wq = np.asarray(Wq, dtype=np.float32)
    Wk = np.asarray(Wk, dtype=np.float32)
    Wv = np.asarray(Wv, dtype=np.float32)
    Wo = np.asarray(Wo, dtype=np.float32)
    bq = np.asarray(bq, dtype=np.float32)
    bk = np.asarray(bk, dtype=np.float32)
    bv = np.asarray(bv, dtype=np.float32)
    bo = np.asarray(bo, dtype=np.float32)
    z_table = np.asarray(z_table, dtype=np.float32)

    nc = _build_program()

    xT = np.ascontiguousarray(x.T).astype(BF16_NP)
    binsT = np.clip(
        np.floor(z_matrix.T / MAX_Z * NUM_Z_BINS).astype(np.int32), 0, NUM_Z_BINS - 1
    )
    exp_tab = np.exp(z_table)  # [16, H] fp32
    sclv = np.concatenate([np.full(HD, SCALE, np.float32),
                           np.ones(HD, np.float32)])

    in_maps = []
    for h in range(NCORES):
        sl = slice(h * HD, (h + 1) * HD)
        wt_h = exp_tab[:, h][binsT].astype(FP16_NP)  # [key, query] layout
        in_maps.append({
            "xT": xT,
            "wqk": np.ascontiguousarray(
                np.concatenate([Wq[:, sl], Wk[:, sl]], axis=1)).astype(BF16_NP),
            "wv": np.ascontiguousarray(Wv[:, sl]).astype(BF16_NP),
            "wo": np.ascontiguousarray(Wo[sl, :]).astype(FP16_NP),
            "bqk": np.concatenate([bq[sl], bk[sl]]),
            "sclv": sclv,
            "wt": wt_h,
        })

    res = bass_utils.run_bass_kernel_spmd(
        nc, in_maps, core_ids=list(range(NCORES)), trace=_trace,
    )

    acc = np.zeros((D, N), dtype=np.float64)
    for h in range(NCORES):
        ypT_h = res.results[h]["ypT"].astype(np.float64)
        z_h = res.results[h]["zrow"].astype(np.float64)
        acc += ypT_h / z_h[None, :]
    out = acc.T + (bv @ Wo)[None, :] + bo[None, :]
    out_f32 = out.astype(np.float32)
    if _trace:
        return out_f32, res
    return out_f32
